# revision 15
# baseline (speedup 1.0000x reference)
"""Trainium2 Bass kernel for GQA attention (B=1, S=2048, D=2048, H=16, KVH=4, HD=128).

Strategy (tensor parallel over heads, 8 cores):
  - Core c computes Q heads {2c, 2c+1} and (redundantly with its pair) KV head c//2.
  - Host pre-transposes x -> xT [D, S] so all projections run with the
    contraction dim D on SBUF partitions.
  - Head-dim permutation trick: wq/wk columns are permuted per head to
    [even dims, odd dims] so RoPE becomes two contiguous 64-partition
    halves. q/k permuted identically => scores unchanged; v untouched.
  - q0/q1/k live in one [128, 3, 2048] tile so RoPE is 6 wide DVE ops.
  - v is projected transposed (vT, N=512 matmuls) then flipped to natural
    [s, hd] layout with 16 PE transposes.
  - Attention is computed fully transposed: scoresT [k, q] tiles in quads
    (4 k-tiles per PSUM group -> one exp per quad), PV accumulation with v
    in natural layout, softmax denominators via one tensor_reduce over the
    probs row + a ones-matmul partition reduction, normalization per head
    via a DRAM-bounce broadcast of the reciprocal sums.
  - Output projection per core uses wo rows [256c:256c+256]; partial
    results are ReduceScattered in 4 row-chunks across the 8 cores; the
    host concatenates the shards.
"""

import numpy as np
from contextlib import ExitStack

import concourse.bacc as bacc
import concourse.tile as tile
import concourse.mybir as mybir
from concourse.bass_utils import run_bass_kernel_spmd

S = 2048
D = 2048
H = 16
KVH = 4
HD = 128
NCORES = 8
F32 = mybir.dt.float32
SCALE = float(1.0 / np.sqrt(HD))
NEG = -1e9

# matmul dtype: float32 (exact) or float32r (fast, reduced precision)
MM_DTYPE = F32

_BUILD_CACHE = {}


def _mm(nc, out, lhsT, rhs, start, stop, dtype):
    if dtype != F32:
        lhsT = lhsT.bitcast(dtype)
        rhs = rhs.bitcast(dtype)
    nc.tensor.matmul(out, lhsT, rhs, start=start, stop=stop)


def _emit_body(nc, tc, io, mm_dtype):
    with ExitStack() as ctx:
        # ---------------- constants (live for the whole body) ----------------
        consts = ctx.enter_context(tc.tile_pool(name="consts", bufs=1))
        wo_sb = consts.tile([128, 2, 2048], F32, tag="wo")
        nc.sync.dma_start(out=wo_sb[:], in_=io["wo"].rearrange("(h p) n -> p h n", p=128))
        mask_sb = consts.tile([128, 4, 512], F32, tag="mask")
        nc.sync.dma_start(out=mask_sb[:], in_=io["maskt"][:])
        ident_sb = consts.tile([128, 128], F32, tag="ident")
        nc.sync.dma_start(out=ident_sb[:], in_=io["ident"][:])
        ones_sb = consts.tile([128, 1], F32, tag="ones")
        nc.vector.memset(ones_sb[:], 1.0)

        # persistent activations
        acts = ctx.enter_context(tc.tile_pool(name="acts", bufs=1))
        qkT = acts.tile([128, 3, 2048], F32, tag="qk")  # [.,0,.]=q0 [.,1,.]=q1 [.,2,.]=k
        vT_sb = acts.tile([128, 2048], F32, tag="vT")
        v_sb = acts.tile([128, 16, 128], F32, tag="v")
        attn0 = acts.tile([128, 2048], F32, tag="attn0")
        attn1 = acts.tile([128, 2048], F32, tag="attn1")

        # ---------------- phase 1: QKV projections (all transposed) ----------------
        with tc.tile_pool(name="wq_pool", bufs=1) as wqp, \
             tc.tile_pool(name="xp", bufs=3) as xpool, \
             tc.tile_pool(name="pproj", bufs=1, space="PSUM") as pproj:
            wqkv_sb = wqp.tile([128, 16, 512], F32, tag="wqkv")
            nc.sync.dma_start(out=wqkv_sb[:],
                              in_=io["wqkv"].rearrange("(t p) n -> p t n", p=128))
            for sh in range(2):  # S halves of 1024
                ps_q0 = pproj.tile([128, 1024], F32, tag="psq0")
                ps_q1 = pproj.tile([128, 1024], F32, tag="psq1")
                ps_k = pproj.tile([128, 1024], F32, tag="psk")
                ps_v = pproj.tile([128, 1024], F32, tag="psv")
                for k in range(16):
                    xt = xpool.tile([128, 1024], F32, tag="xt")
                    nc.sync.dma_start(
                        out=xt[:],
                        in_=io["xT"][k * 128:(k + 1) * 128, sh * 1024:(sh + 1) * 1024],
                    )
                    st = k == 0
                    sp = k == 15
                    for n in range(2):
                        nsl = slice(n * 512, (n + 1) * 512)
                        _mm(nc, ps_q0[:, nsl], wqkv_sb[:, k, 0:128], xt[:, nsl], st, sp, mm_dtype)
                        _mm(nc, ps_q1[:, nsl], wqkv_sb[:, k, 128:256], xt[:, nsl], st, sp, mm_dtype)
                        _mm(nc, ps_k[:, nsl], wqkv_sb[:, k, 256:384], xt[:, nsl], st, sp, mm_dtype)
                        _mm(nc, ps_v[:, nsl], wqkv_sb[:, k, 384:512], xt[:, nsl], st, sp, mm_dtype)
                ssl = slice(sh * 1024, (sh + 1) * 1024)
                nc.vector.tensor_copy(qkT[:, 0, ssl], ps_q0[:])
                nc.vector.tensor_copy(qkT[:, 1, ssl], ps_q1[:])
                nc.vector.tensor_copy(qkT[:, 2, ssl], ps_k[:])
                nc.vector.tensor_copy(vT_sb[:, ssl], ps_v[:])

        # ---------------- phase 1.3: vT -> v via PE transposes ----------------
        with tc.tile_pool(name="ptr", bufs=2, space="PSUM") as ptr:
            for j4 in range(4):
                tp4 = ptr.tile([128, 4, 128], F32, tag="tp4")
                for i in range(4):
                    j = 4 * j4 + i
                    nc.tensor.transpose(tp4[:, i, :],
                                        vT_sb[:, j * 128:(j + 1) * 128], ident_sb[:])
                nc.vector.tensor_copy(v_sb[:, 4 * j4:4 * j4 + 4, :], tp4[:])

        # ---------------- phase 1.5: RoPE on q0, q1, k (in place) ----------------
        # DVE ops are lane-locked, so bring the odd halves (partitions 64-127)
        # down to partitions 0-63 via DMA, rotate there, and DMA o1 back up.
        with tc.tile_pool(name="rope", bufs=1) as rp, \
             tc.tile_pool(name="fr", bufs=1) as fr:
            cos_sb = fr.tile([64, 2048], F32, tag="cos")
            nc.sync.dma_start(out=cos_sb[:], in_=io["cosT"][:])
            sin_sb = fr.tile([64, 2048], F32, tag="sin")
            nc.sync.dma_start(out=sin_sb[:], in_=io["sinT"][:])
            cos_b = cos_sb[:].unsqueeze(1).broadcast_to((64, 3, 2048))
            sin_b = sin_sb[:].unsqueeze(1).broadcast_to((64, 3, 2048))
            t0 = qkT[0:64, :, :]
            t1c = rp.tile([64, 3, 2048], F32, tag="t1c")
            nc.sync.dma_start(out=t1c[:], in_=qkT[64:128, :, :])
            o1 = rp.tile([64, 3, 2048], F32, tag="o1")
            tmp = rp.tile([64, 3, 2048], F32, tag="tmp")
            nc.vector.tensor_mul(o1[:], t0, sin_b)
            nc.vector.tensor_mul(tmp[:], t1c[:], cos_b)
            nc.vector.tensor_add(o1[:], o1[:], tmp[:])
            nc.vector.tensor_mul(tmp[:], t1c[:], sin_b)
            nc.vector.tensor_mul(t1c[:], t0, cos_b)
            nc.vector.tensor_sub(qkT[0:64, :, :], t1c[:], tmp[:])
            nc.sync.dma_start(out=qkT[64:128, :, :], in_=o1[:])

        # ---------------- phase 2: attention ----------------
        patt = ctx.enter_context(tc.tile_pool(name="patt", bufs=1, space="PSUM"))
        spool = ctx.enter_context(tc.tile_pool(name="satt", bufs=1))
        sp2 = ctx.enter_context(tc.tile_pool(name="sp2", bufs=2))
        dram = ctx.enter_context(tc.tile_pool(name="dram", bufs=1, space="DRAM"))
        opool = ctx.enter_context(tc.tile_pool(name="osb", bufs=1))

        for h, attnT in enumerate((attn0, attn1)):
            recip_h = sp2.tile([1, 2048], F32, tag="reciph")
            for qc in range(4):
                qsl = slice(qc * 512, (qc + 1) * 512)
                ps_o = patt.tile([128, 512], F32, tag="ps_o")
                pt = spool.tile([128, 16, 512], F32, tag="pt")
                nq = qc + 1  # quads of k-tiles
                for g in range(nq):
                    j0 = 4 * g
                    ps_quad = patt.tile([128, 4, 512], F32, tag="pbig")
                    for i in range(4):
                        _mm(nc, ps_quad[:, i, :],
                            qkT[:, 2, (j0 + i) * 128:(j0 + i + 1) * 128],
                            qkT[:, h, qsl], True, True, mm_dtype)
                    if g == nq - 1:  # boundary quad: apply causal mask tiles
                        nc.vector.tensor_add(ps_quad[:], ps_quad[:], mask_sb[:])
                    nc.scalar.activation(pt[:, j0:j0 + 4, :], ps_quad[:],
                                         mybir.ActivationFunctionType.Exp, scale=SCALE)
                    for i in range(4):
                        _mm(nc, ps_o[:], v_sb[:, j0 + i, :], pt[:, j0 + i, :],
                            j0 + i == 0, j0 + i == 4 * nq - 1, mm_dtype)
                nc.vector.tensor_copy(attnT[:, qsl], ps_o[:])
                # softmax denominators: DVE-reduce probs over k-tiles, then a
                # ones-matmul partition reduction
                njt = 4 * nq
                red = sp2.tile([128, 512], F32, tag="red")
                nc.vector.tensor_reduce(
                    red[:], pt[:, 0:njt, :].rearrange("p a b -> p b a"),
                    axis=mybir.AxisListType.X, op=mybir.AluOpType.add)
                ps_cs = patt.tile([1, 512], F32, tag="ps_cs")
                _mm(nc, ps_cs[:], ones_sb[:], red[:], True, True, mm_dtype)
                nc.vector.reciprocal(recip_h[:, qsl], ps_cs[:])
            # normalize the whole head at once via a DRAM-bounce broadcast
            rb = dram.tile([1, 2048], F32, name=f"rb_{h}")
            nc.sync.dma_start(out=rb[:], in_=recip_h[:])
            bc = sp2.tile([128, 2048], F32, tag="bc")
            nc.sync.dma_start(out=bc[:], in_=rb.to_broadcast((128, 2048)))
            nc.vector.tensor_mul(attnT[:], attnT[:], bc[:])

        # ---------------- phase 3: output projection + ReduceScatter ----------------
        # 4 row-chunk collectives so the ReduceScatter overlaps the remaining
        # output-projection compute instead of being fully exposed at the end.
        for q4 in range(4):
            woacc = dram.tile([512, 2048], F32, name=f"woacc_{q4}")
            woacc_v = woacc.rearrange("(a p) n -> p a n", p=128)  # [128, 4, 2048]
            o_sb = opool.tile([128, 4, 2048], F32, tag="osb")
            for sti in range(4):
                st = 4 * q4 + sti
                ssl = slice(st * 128, (st + 1) * 128)
                ps_wo = patt.tile([128, 2048], F32, tag="pbig")
                for n in range(4):
                    nsl = slice(n * 512, (n + 1) * 512)
                    _mm(nc, ps_wo[:, nsl], attn0[:, ssl], wo_sb[:, 0, nsl],
                        True, False, mm_dtype)
                    _mm(nc, ps_wo[:, nsl], attn1[:, ssl], wo_sb[:, 1, nsl],
                        False, True, mm_dtype)
                nc.vector.tensor_copy(o_sb[:, sti, :], ps_wo[:])
            nc.sync.dma_start(out=woacc_v[:], in_=o_sb[:])

            rs_out = dram.tile([64, 2048], F32, name=f"rsout_{q4}")
            nc.gpsimd.collective_compute(
                "ReduceScatter",
                mybir.AluOpType.add,
                replica_groups=[list(range(NCORES))],
                ins=[woacc.opt()],
                outs=[rs_out.opt()],
            )
            nc.sync.dma_start(out=io["out"][q4 * 64:(q4 + 1) * 64, :], in_=rs_out[:])


def build(mm_dtype=None, repeat=1, num_devices=NCORES):
    mm_dtype = mm_dtype or MM_DTYPE
    key = (str(mm_dtype), repeat, num_devices)
    if key in _BUILD_CACHE:
        return _BUILD_CACHE[key]
    nc = bacc.Bacc("TRN2", target_bir_lowering=False, debug=False,
                   num_devices=num_devices)
    io = {
        "xT": nc.dram_tensor("xT", [D, S], F32, kind="ExternalInput").ap(),
        "wqkv": nc.dram_tensor("wqkv", [D, 512], F32, kind="ExternalInput").ap(),
        "wo": nc.dram_tensor("wo", [256, D], F32, kind="ExternalInput").ap(),
        "cosT": nc.dram_tensor("cosT", [64, S], F32, kind="ExternalInput").ap(),
        "sinT": nc.dram_tensor("sinT", [64, S], F32, kind="ExternalInput").ap(),
        "maskt": nc.dram_tensor("maskt", [128, 4, 512], F32, kind="ExternalInput").ap(),
        "ident": nc.dram_tensor("ident", [128, 128], F32, kind="ExternalInput").ap(),
        "out": nc.dram_tensor("out", [S // NCORES, D], F32, kind="ExternalOutput").ap(),
    }
    with tile.TileContext(nc) as tc:
        for _ in range(repeat):
            _emit_body(nc, tc, io, mm_dtype)
    nc.compile()
    _BUILD_CACHE[key] = nc
    return nc


def prepare_in_maps(x, wq, wk, wv, wo, freqs_cos, freqs_sin):
    x2d = np.asarray(x, dtype=np.float32).reshape(S, D)
    xT = np.ascontiguousarray(x2d.T)
    cosT = np.ascontiguousarray(np.asarray(freqs_cos, np.float32).T)
    sinT = np.ascontiguousarray(np.asarray(freqs_sin, np.float32).T)

    # even dims first, then odd dims (applied to q and k only)
    perm = np.concatenate([np.arange(0, HD, 2), np.arange(1, HD, 2)])

    # boundary mask tiles in scoresT layout: visible iff jl <= ql - 128*r
    jl = np.arange(128)[:, None]
    ql = np.arange(512)[None, :]
    maskt = np.empty((128, 4, 512), np.float32)
    for r in range(4):
        maskt[:, r, :] = np.where(jl <= ql - 128 * r, 0.0, NEG)

    ident = np.eye(128, dtype=np.float32)

    wq = np.asarray(wq, np.float32)
    wk = np.asarray(wk, np.float32)
    wv = np.asarray(wv, np.float32)
    wo = np.asarray(wo, np.float32)

    in_maps = []
    for c in range(NCORES):
        g = c // 2
        wq_c = wq[:, 2 * c * HD:(2 * c + 2) * HD].reshape(D, 2, HD)[:, :, perm].reshape(D, 2 * HD)
        wk_c = wk[:, g * HD:(g + 1) * HD][:, perm]
        wv_c = wv[:, g * HD:(g + 1) * HD]
        wqkv_c = np.ascontiguousarray(
            np.concatenate([wq_c, wk_c, wv_c], axis=1))  # [D, 512]
        wo_c = np.ascontiguousarray(wo[2 * c * HD:(2 * c + 2) * HD, :])  # [256, D]
        in_maps.append({
            "xT": xT,
            "wqkv": wqkv_c,
            "wo": wo_c,
            "cosT": cosT,
            "sinT": sinT,
            "maskt": maskt,
            "ident": ident,
        })
    return in_maps


def assemble_output(results):
    # RS chunk q4 covers global rows [512*q4, 512*q4+512); core c holds
    # rows [512*q4 + 64*c, 512*q4 + 64*c + 64) at out[64*q4 : 64*q4+64].
    shards = np.stack([results[c]["out"].reshape(4, 64, D) for c in range(NCORES)],
                      axis=1)  # [q4, core, 64, D]
    return np.ascontiguousarray(shards.reshape(S, D)).reshape(1, S, D)


def kernel(x, wq, wk, wv, wo, freqs_cos, freqs_sin, mask):
    nc = build()
    in_maps = prepare_in_maps(x, wq, wk, wv, wo, freqs_cos, freqs_sin)
    res = run_bass_kernel_spmd(nc, in_maps, core_ids=list(range(NCORES)))
    return assemble_output(res.results).astype(np.float32)


# revision 16
# speedup vs baseline: 1.6333x; 1.6333x over previous
"""Trainium2 Bass kernel for GQA attention (B=1, S=2048, D=2048, H=16, KVH=4, HD=128).

Strategy (tensor parallel over heads, 8 cores):
  - Core c computes Q heads {2c, 2c+1} and (redundantly with its pair) KV head c//2.
  - Host pre-transposes x -> xT [D, S] so all projections run with the
    contraction dim D on SBUF partitions.
  - Head-dim permutation trick: wq/wk columns are permuted per head to
    [even dims, odd dims] so RoPE becomes two contiguous 64-partition
    halves. q/k permuted identically => scores unchanged; v untouched.
  - q0/q1/k live in one [128, 3, 2048] tile so RoPE is 6 wide DVE ops.
  - v is projected transposed (vT, N=512 matmuls) then flipped to natural
    [s, hd] layout with 16 PE transposes.
  - Attention is computed fully transposed: scoresT [k, q] tiles in quads
    (4 k-tiles per PSUM group -> one exp per quad), PV accumulation with v
    in natural layout, softmax denominators via one tensor_reduce over the
    probs row + a ones-matmul partition reduction, normalization per head
    via a DRAM-bounce broadcast of the reciprocal sums.
  - Output projection per core uses wo rows [256c:256c+256]; partial
    results are ReduceScattered in 4 row-chunks across the 8 cores; the
    host concatenates the shards.
"""

import numpy as np
from contextlib import ExitStack

import concourse.bacc as bacc
import concourse.tile as tile
import concourse.mybir as mybir
from concourse.bass_utils import run_bass_kernel_spmd

S = 2048
D = 2048
H = 16
KVH = 4
HD = 128
NCORES = 8
F32 = mybir.dt.float32
SCALE = float(1.0 / np.sqrt(HD))
NEG = -1e9

# matmul dtype: float32 (exact) or float32r (fast, reduced precision)
MM_DTYPE = F32

_BUILD_CACHE = {}


def _mm(nc, out, lhsT, rhs, start, stop, dtype):
    if dtype != F32:
        lhsT = lhsT.bitcast(dtype)
        rhs = rhs.bitcast(dtype)
    nc.tensor.matmul(out, lhsT, rhs, start=start, stop=stop)


def _emit_body(nc, tc, io, mm_dtype):
    with ExitStack() as ctx:
        # ---------------- constants (live for the whole body) ----------------
        consts = ctx.enter_context(tc.tile_pool(name="consts", bufs=1))
        wo_sb = consts.tile([128, 2, 2048], F32, tag="wo")
        nc.sync.dma_start(out=wo_sb[:], in_=io["wo"].rearrange("(h p) n -> p h n", p=128))
        mask_sb = consts.tile([128, 4, 512], F32, tag="mask")
        nc.sync.dma_start(out=mask_sb[:], in_=io["maskt"][:])
        ident_sb = consts.tile([128, 128], F32, tag="ident")
        nc.sync.dma_start(out=ident_sb[:], in_=io["ident"][:])
        ones_sb = consts.tile([128, 1], F32, tag="ones")
        nc.vector.memset(ones_sb[:], 1.0)

        # persistent activations
        acts = ctx.enter_context(tc.tile_pool(name="acts", bufs=1))
        qkT = acts.tile([128, 3, 2048], F32, tag="qk")  # [.,0,.]=q0 [.,1,.]=q1 [.,2,.]=k
        vT_sb = acts.tile([128, 2048], F32, tag="vT")
        v_sb = acts.tile([128, 16, 128], F32, tag="v")
        attn0 = acts.tile([128, 2048], F32, tag="attn0")
        attn1 = acts.tile([128, 2048], F32, tag="attn1")

        # ---------------- phase 1: QKV projections (all transposed) ----------------
        with tc.tile_pool(name="wq_pool", bufs=1) as wqp, \
             tc.tile_pool(name="xp", bufs=3) as xpool, \
             tc.tile_pool(name="pproj", bufs=1, space="PSUM") as pproj:
            wqkv_sb = wqp.tile([128, 16, 512], F32, tag="wqkv")
            nc.sync.dma_start(out=wqkv_sb[:],
                              in_=io["wqkv"].rearrange("(t p) n -> p t n", p=128))
            for sh in range(2):  # S halves of 1024
                ps_q0 = pproj.tile([128, 1024], F32, tag="psq0")
                ps_q1 = pproj.tile([128, 1024], F32, tag="psq1")
                ps_k = pproj.tile([128, 1024], F32, tag="psk")
                ps_v = pproj.tile([128, 1024], F32, tag="psv")
                for k in range(16):
                    xt = xpool.tile([128, 1024], F32, tag="xt")
                    nc.sync.dma_start(
                        out=xt[:],
                        in_=io["xT"][k * 128:(k + 1) * 128, sh * 1024:(sh + 1) * 1024],
                    )
                    st = k == 0
                    sp = k == 15
                    for n in range(2):
                        nsl = slice(n * 512, (n + 1) * 512)
                        _mm(nc, ps_q0[:, nsl], wqkv_sb[:, k, 0:128], xt[:, nsl], st, sp, mm_dtype)
                        _mm(nc, ps_q1[:, nsl], wqkv_sb[:, k, 128:256], xt[:, nsl], st, sp, mm_dtype)
                        _mm(nc, ps_k[:, nsl], wqkv_sb[:, k, 256:384], xt[:, nsl], st, sp, mm_dtype)
                        _mm(nc, ps_v[:, nsl], wqkv_sb[:, k, 384:512], xt[:, nsl], st, sp, mm_dtype)
                ssl = slice(sh * 1024, (sh + 1) * 1024)
                nc.vector.tensor_copy(qkT[:, 0, ssl], ps_q0[:])
                nc.vector.tensor_copy(qkT[:, 1, ssl], ps_q1[:])
                nc.vector.tensor_copy(qkT[:, 2, ssl], ps_k[:])
                nc.vector.tensor_copy(vT_sb[:, ssl], ps_v[:])

        # ---------------- phase 1.3: vT -> v via PE transposes ----------------
        patt = ctx.enter_context(tc.tile_pool(name="patt", bufs=1, space="PSUM"))
        if True:
            for j4 in range(4):
                tp4 = patt.tile([128, 4, 128], F32, tag="tp4")
                for i in range(4):
                    j = 4 * j4 + i
                    nc.tensor.transpose(tp4[:, i, :],
                                        vT_sb[:, j * 128:(j + 1) * 128], ident_sb[:])
                nc.vector.tensor_copy(v_sb[:, 4 * j4:4 * j4 + 4, :], tp4[:])

        # ---------------- phase 1.5: RoPE on q0, q1, k (in place) ----------------
        # DVE ops are lane-locked, so bring the odd halves (partitions 64-127)
        # down to partitions 0-63 via DMA, rotate there, and DMA o1 back up.
        with tc.tile_pool(name="rope", bufs=1) as rp:
            cos_sb = rp.tile([64, 2048], F32, tag="cos")
            nc.sync.dma_start(out=cos_sb[:], in_=io["cosT"][:])
            sin_sb = rp.tile([64, 2048], F32, tag="sin")
            nc.sync.dma_start(out=sin_sb[:], in_=io["sinT"][:])
            cos_b = cos_sb[:].unsqueeze(1).broadcast_to((64, 3, 2048))
            sin_b = sin_sb[:].unsqueeze(1).broadcast_to((64, 3, 2048))
            t0 = qkT[0:64, :, :]
            t1c = rp.tile([64, 3, 2048], F32, tag="t1c")
            nc.sync.dma_start(out=t1c[:], in_=qkT[64:128, :, :])
            o1 = rp.tile([64, 3, 2048], F32, tag="o1")
            tmp = rp.tile([64, 3, 2048], F32, tag="tmp")
            nc.vector.tensor_mul(o1[:], t0, sin_b)
            nc.vector.tensor_mul(tmp[:], t1c[:], cos_b)
            nc.vector.tensor_add(o1[:], o1[:], tmp[:])
            nc.vector.tensor_mul(tmp[:], t1c[:], sin_b)
            nc.vector.tensor_mul(t1c[:], t0, cos_b)
            nc.vector.tensor_sub(qkT[0:64, :, :], t1c[:], tmp[:])
            nc.sync.dma_start(out=qkT[64:128, :, :], in_=o1[:])

        # ---------------- phase 2: attention ----------------
        spool = ctx.enter_context(tc.tile_pool(name="satt", bufs=1))
        sp2 = ctx.enter_context(tc.tile_pool(name="sp2", bufs=2))
        dram = ctx.enter_context(tc.tile_pool(name="dram", bufs=1, space="DRAM"))
        opool = spool

        for h, attnT in enumerate((attn0, attn1)):
            recip_h = sp2.tile([1, 2048], F32, tag="reciph")
            for qc in range(4):
                qsl = slice(qc * 512, (qc + 1) * 512)
                ps_o = patt.tile([128, 512], F32, tag="ps_o")
                pt = spool.tile([128, 16, 512], F32, tag="pt")
                nq = qc + 1  # quads of k-tiles
                for g in range(nq):
                    j0 = 4 * g
                    ps_quad = patt.tile([128, 4, 512], F32, tag="pbig")
                    for i in range(4):
                        _mm(nc, ps_quad[:, i, :],
                            qkT[:, 2, (j0 + i) * 128:(j0 + i + 1) * 128],
                            qkT[:, h, qsl], True, True, mm_dtype)
                    if g == nq - 1:  # boundary quad: apply causal mask tiles
                        nc.vector.tensor_add(ps_quad[:], ps_quad[:], mask_sb[:])
                    nc.scalar.activation(pt[:, j0:j0 + 4, :], ps_quad[:],
                                         mybir.ActivationFunctionType.Exp, scale=SCALE)
                    for i in range(4):
                        _mm(nc, ps_o[:], v_sb[:, j0 + i, :], pt[:, j0 + i, :],
                            j0 + i == 0, j0 + i == 4 * nq - 1, mm_dtype)
                nc.vector.tensor_copy(attnT[:, qsl], ps_o[:])
                # softmax denominators: DVE-reduce probs over k-tiles, then a
                # ones-matmul partition reduction
                njt = 4 * nq
                red = sp2.tile([128, 512], F32, tag="red")
                nc.vector.tensor_reduce(
                    red[:], pt[:, 0:njt, :].rearrange("p a b -> p b a"),
                    axis=mybir.AxisListType.X, op=mybir.AluOpType.add)
                ps_cs = patt.tile([1, 512], F32, tag="ps_cs")
                _mm(nc, ps_cs[:], ones_sb[:], red[:], True, True, mm_dtype)
                nc.vector.reciprocal(recip_h[:, qsl], ps_cs[:])
            # normalize the whole head at once via a DRAM-bounce broadcast
            rb = dram.tile([1, 2048], F32, name=f"rb_{h}")
            nc.sync.dma_start(out=rb[:], in_=recip_h[:])
            bc = sp2.tile([128, 2048], F32, tag="bc")
            nc.sync.dma_start(out=bc[:], in_=rb.to_broadcast((128, 2048)))
            nc.vector.tensor_mul(attnT[:], attnT[:], bc[:])

        # ---------------- phase 3: output projection + ReduceScatter ----------------
        # 4 row-chunk collectives so the ReduceScatter overlaps the remaining
        # output-projection compute instead of being fully exposed at the end.
        for q4 in range(4):
            woacc = dram.tile([512, 2048], F32, name=f"woacc_{q4}")
            woacc_v = woacc.rearrange("(a p) n -> p a n", p=128)  # [128, 4, 2048]
            o_sb = opool.tile([128, 4, 2048], F32, tag="osb")
            for sti in range(4):
                st = 4 * q4 + sti
                ssl = slice(st * 128, (st + 1) * 128)
                ps_wo = patt.tile([128, 2048], F32, tag="pbig")
                for n in range(4):
                    nsl = slice(n * 512, (n + 1) * 512)
                    _mm(nc, ps_wo[:, nsl], attn0[:, ssl], wo_sb[:, 0, nsl],
                        True, False, mm_dtype)
                    _mm(nc, ps_wo[:, nsl], attn1[:, ssl], wo_sb[:, 1, nsl],
                        False, True, mm_dtype)
                nc.vector.tensor_copy(o_sb[:, sti, :], ps_wo[:])
            nc.sync.dma_start(out=woacc_v[:], in_=o_sb[:])

            rs_out = dram.tile([64, 2048], F32, name=f"rsout_{q4}")
            nc.gpsimd.collective_compute(
                "ReduceScatter",
                mybir.AluOpType.add,
                replica_groups=[list(range(NCORES))],
                ins=[woacc.opt()],
                outs=[rs_out.opt()],
            )
            nc.sync.dma_start(out=io["out"][q4 * 64:(q4 + 1) * 64, :], in_=rs_out[:])


def build(mm_dtype=None, repeat=1, num_devices=NCORES):
    mm_dtype = mm_dtype or MM_DTYPE
    key = (str(mm_dtype), repeat, num_devices)
    if key in _BUILD_CACHE:
        return _BUILD_CACHE[key]
    nc = bacc.Bacc("TRN2", target_bir_lowering=False, debug=False,
                   num_devices=num_devices)
    io = {
        "xT": nc.dram_tensor("xT", [D, S], F32, kind="ExternalInput").ap(),
        "wqkv": nc.dram_tensor("wqkv", [D, 512], F32, kind="ExternalInput").ap(),
        "wo": nc.dram_tensor("wo", [256, D], F32, kind="ExternalInput").ap(),
        "cosT": nc.dram_tensor("cosT", [64, S], F32, kind="ExternalInput").ap(),
        "sinT": nc.dram_tensor("sinT", [64, S], F32, kind="ExternalInput").ap(),
        "maskt": nc.dram_tensor("maskt", [128, 4, 512], F32, kind="ExternalInput").ap(),
        "ident": nc.dram_tensor("ident", [128, 128], F32, kind="ExternalInput").ap(),
        "out": nc.dram_tensor("out", [S // NCORES, D], F32, kind="ExternalOutput").ap(),
    }
    with tile.TileContext(nc) as tc:
        for _ in range(repeat):
            _emit_body(nc, tc, io, mm_dtype)
    nc.compile()
    _BUILD_CACHE[key] = nc
    return nc


def prepare_in_maps(x, wq, wk, wv, wo, freqs_cos, freqs_sin):
    x2d = np.asarray(x, dtype=np.float32).reshape(S, D)
    xT = np.ascontiguousarray(x2d.T)
    cosT = np.ascontiguousarray(np.asarray(freqs_cos, np.float32).T)
    sinT = np.ascontiguousarray(np.asarray(freqs_sin, np.float32).T)

    # even dims first, then odd dims (applied to q and k only)
    perm = np.concatenate([np.arange(0, HD, 2), np.arange(1, HD, 2)])

    # boundary mask tiles in scoresT layout: visible iff jl <= ql - 128*r
    jl = np.arange(128)[:, None]
    ql = np.arange(512)[None, :]
    maskt = np.empty((128, 4, 512), np.float32)
    for r in range(4):
        maskt[:, r, :] = np.where(jl <= ql - 128 * r, 0.0, NEG)

    ident = np.eye(128, dtype=np.float32)

    wq = np.asarray(wq, np.float32)
    wk = np.asarray(wk, np.float32)
    wv = np.asarray(wv, np.float32)
    wo = np.asarray(wo, np.float32)

    in_maps = []
    for c in range(NCORES):
        g = c // 2
        wq_c = wq[:, 2 * c * HD:(2 * c + 2) * HD].reshape(D, 2, HD)[:, :, perm].reshape(D, 2 * HD)
        wk_c = wk[:, g * HD:(g + 1) * HD][:, perm]
        wv_c = wv[:, g * HD:(g + 1) * HD]
        wqkv_c = np.ascontiguousarray(
            np.concatenate([wq_c, wk_c, wv_c], axis=1))  # [D, 512]
        wo_c = np.ascontiguousarray(wo[2 * c * HD:(2 * c + 2) * HD, :])  # [256, D]
        in_maps.append({
            "xT": xT,
            "wqkv": wqkv_c,
            "wo": wo_c,
            "cosT": cosT,
            "sinT": sinT,
            "maskt": maskt,
            "ident": ident,
        })
    return in_maps


def assemble_output(results):
    # RS chunk q4 covers global rows [512*q4, 512*q4+512); core c holds
    # rows [512*q4 + 64*c, 512*q4 + 64*c + 64) at out[64*q4 : 64*q4+64].
    shards = np.stack([results[c]["out"].reshape(4, 64, D) for c in range(NCORES)],
                      axis=1)  # [q4, core, 64, D]
    return np.ascontiguousarray(shards.reshape(S, D)).reshape(1, S, D)


def kernel(x, wq, wk, wv, wo, freqs_cos, freqs_sin, mask):
    nc = build()
    in_maps = prepare_in_maps(x, wq, wk, wv, wo, freqs_cos, freqs_sin)
    res = run_bass_kernel_spmd(nc, in_maps, core_ids=list(range(NCORES)))
    return assemble_output(res.results).astype(np.float32)


# revision 17
# speedup vs baseline: 7.1250x; 4.3623x over previous
"""Trainium2 Bass kernel for GQA attention (B=1, S=2048, D=2048, H=16, KVH=4, HD=128).

Strategy (tensor parallel over heads, 8 cores):
  - Core c computes Q heads {2c, 2c+1} and (redundantly with its pair) KV head c//2.
  - Host pre-transposes x -> xT [D, S] so all projections run with the
    contraction dim D on SBUF partitions.
  - Head-dim permutation trick: wq/wk columns are permuted per head to
    [even dims, odd dims] so RoPE becomes two contiguous 64-partition
    halves. q/k permuted identically => scores unchanged; v untouched.
  - q0/q1/k live in one [128, 3, 2048] tile so RoPE is 6 wide DVE ops.
  - v is projected transposed (vT, N=512 matmuls) then flipped to natural
    [s, hd] layout with 16 PE transposes.
  - Attention is computed fully transposed: scoresT [k, q] tiles in quads
    (4 k-tiles per PSUM group -> one exp per quad), PV accumulation with v
    in natural layout, softmax denominators via one tensor_reduce over the
    probs row + a ones-matmul partition reduction, normalization per head
    via a DRAM-bounce broadcast of the reciprocal sums.
  - Output projection per core uses wo rows [256c:256c+256]; partial
    results are ReduceScattered in 4 row-chunks across the 8 cores; the
    host concatenates the shards.
"""

import numpy as np
from contextlib import ExitStack

import concourse.bacc as bacc
import concourse.tile as tile
import concourse.mybir as mybir
from concourse.bass_utils import run_bass_kernel_spmd

S = 2048
D = 2048
H = 16
KVH = 4
HD = 128
NCORES = 8
F32 = mybir.dt.float32
SCALE = float(1.0 / np.sqrt(HD))
NEG = -1e9

# matmul dtype: float32 (exact) or float32r (fast, reduced precision)
MM_DTYPE = F32

_BUILD_CACHE = {}


def _mm(nc, out, lhsT, rhs, start, stop, dtype):
    if dtype != F32:
        lhsT = lhsT.bitcast(dtype)
        rhs = rhs.bitcast(dtype)
    nc.tensor.matmul(out, lhsT, rhs, start=start, stop=stop)


def _emit_body(nc, tc, io, mm_dtype):
    with ExitStack() as ctx:
        # ---------------- constants (live for the whole body) ----------------
        consts = ctx.enter_context(tc.tile_pool(name="consts", bufs=1))
        wo_sb = consts.tile([128, 2, 2048], F32, tag="wo")
        nc.sync.dma_start(out=wo_sb[:], in_=io["wo"].rearrange("(h p) n -> p h n", p=128))
        mask_sb = consts.tile([128, 4, 512], F32, tag="mask")
        nc.sync.dma_start(out=mask_sb[:], in_=io["maskt"][:])
        ident_sb = consts.tile([128, 128], F32, tag="ident")
        nc.sync.dma_start(out=ident_sb[:], in_=io["ident"][:])
        ones_sb = consts.tile([128, 1], F32, tag="ones")
        nc.vector.memset(ones_sb[:], 1.0)

        # persistent activations
        acts = ctx.enter_context(tc.tile_pool(name="acts", bufs=1))
        qkT = acts.tile([128, 3, 2048], F32, tag="qk")  # [.,0,.]=q0 [.,1,.]=q1 [.,2,.]=k
        vT_sb = acts.tile([128, 2048], F32, tag="vT")
        v_sb = acts.tile([128, 16, 128], F32, tag="v")
        attn0 = acts.tile([128, 2048], F32, tag="attn0")
        attn1 = acts.tile([128, 2048], F32, tag="attn1")

        # ---------------- phase 1: QKV projections (all transposed) ----------------
        with tc.tile_pool(name="wq_pool", bufs=1) as wqp, \
             tc.tile_pool(name="xp", bufs=3) as xpool, \
             tc.tile_pool(name="pproj", bufs=1, space="PSUM") as pproj:
            wqkv_sb = wqp.tile([128, 16, 512], F32, tag="wqkv")
            nc.sync.dma_start(out=wqkv_sb[:],
                              in_=io["wqkv"].rearrange("(t p) n -> p t n", p=128))
            for sh in range(2):  # S halves of 1024
                ps_q0 = pproj.tile([128, 1024], F32, tag="psq0")
                ps_q1 = pproj.tile([128, 1024], F32, tag="psq1")
                ps_k = pproj.tile([128, 1024], F32, tag="psk")
                ps_v = pproj.tile([128, 1024], F32, tag="psv")
                for k in range(16):
                    xt = xpool.tile([128, 1024], F32, tag="xt")
                    nc.sync.dma_start(
                        out=xt[:],
                        in_=io["xT"][k * 128:(k + 1) * 128, sh * 1024:(sh + 1) * 1024],
                    )
                    st = k == 0
                    sp = k == 15
                    for n in range(2):
                        nsl = slice(n * 512, (n + 1) * 512)
                        _mm(nc, ps_q0[:, nsl], wqkv_sb[:, k, 0:128], xt[:, nsl], st, sp, mm_dtype)
                        _mm(nc, ps_q1[:, nsl], wqkv_sb[:, k, 128:256], xt[:, nsl], st, sp, mm_dtype)
                        _mm(nc, ps_k[:, nsl], wqkv_sb[:, k, 256:384], xt[:, nsl], st, sp, mm_dtype)
                        _mm(nc, ps_v[:, nsl], wqkv_sb[:, k, 384:512], xt[:, nsl], st, sp, mm_dtype)
                ssl = slice(sh * 1024, (sh + 1) * 1024)
                nc.vector.tensor_copy(qkT[:, 0, ssl], ps_q0[:])
                nc.vector.tensor_copy(qkT[:, 1, ssl], ps_q1[:])
                nc.vector.tensor_copy(qkT[:, 2, ssl], ps_k[:])
                nc.vector.tensor_copy(vT_sb[:, ssl], ps_v[:])

        # ---------------- phase 1.3: vT -> v via PE transposes ----------------
        patt = ctx.enter_context(tc.tile_pool(name="patt", bufs=1, space="PSUM"))
        for j4 in range(4):
            tp4 = patt.tile([128, 4, 128], F32, tag="tp4")
            for i in range(4):
                j = 4 * j4 + i
                nc.tensor.transpose(tp4[:, i, :],
                                    vT_sb[:, j * 128:(j + 1) * 128], ident_sb[:])
            nc.vector.tensor_copy(v_sb[:, 4 * j4:4 * j4 + 4, :], tp4[:])

        # ---------------- phase 1.5: RoPE on q0, q1, k (in place) ----------------
        # DVE ops are lane-locked, so bring the odd halves (partitions 64-127)
        # down to partitions 0-63 via DMA, rotate there, and DMA o1 back up.
        with tc.tile_pool(name="rope", bufs=1) as rp:
            cos_sb = rp.tile([64, 2048], F32, tag="cos")
            nc.sync.dma_start(out=cos_sb[:], in_=io["cosT"][:])
            sin_sb = rp.tile([64, 2048], F32, tag="sin")
            nc.sync.dma_start(out=sin_sb[:], in_=io["sinT"][:])
            cos_b = cos_sb[:].unsqueeze(1).broadcast_to((64, 3, 2048))
            sin_b = sin_sb[:].unsqueeze(1).broadcast_to((64, 3, 2048))
            t0 = qkT[0:64, :, :]
            t1c = rp.tile([64, 3, 2048], F32, tag="t1c")
            nc.sync.dma_start(out=t1c[:], in_=qkT[64:128, :, :])
            o1 = rp.tile([64, 3, 2048], F32, tag="o1")
            tmp = rp.tile([64, 3, 2048], F32, tag="tmp")
            nc.vector.tensor_mul(o1[:], t0, sin_b)
            nc.vector.tensor_mul(tmp[:], t1c[:], cos_b)
            nc.vector.tensor_add(o1[:], o1[:], tmp[:])
            nc.vector.tensor_mul(tmp[:], t1c[:], sin_b)
            nc.vector.tensor_mul(t1c[:], t0, cos_b)
            nc.vector.tensor_sub(qkT[0:64, :, :], t1c[:], tmp[:])
            nc.sync.dma_start(out=qkT[64:128, :, :], in_=o1[:])

        # ---------------- phase 2: attention ----------------
        spool = ctx.enter_context(tc.tile_pool(name="satt", bufs=1))
        sp2 = ctx.enter_context(tc.tile_pool(name="sp2", bufs=2))
        dram = ctx.enter_context(tc.tile_pool(name="dram", bufs=1, space="DRAM"))
        opool = spool

        for h, attnT in enumerate((attn0, attn1)):
            recip_h = sp2.tile([1, 2048], F32, tag="reciph")
            for qc in range(4):
                qsl = slice(qc * 512, (qc + 1) * 512)
                ps_o = patt.tile([128, 512], F32, tag="ps_o")
                pt = spool.tile([128, 16, 512], F32, tag="pt")
                nq = qc + 1  # quads of k-tiles
                for g in range(nq):
                    j0 = 4 * g
                    ps_quad = patt.tile([128, 4, 512], F32, tag="pbig")
                    for i in range(4):
                        _mm(nc, ps_quad[:, i, :],
                            qkT[:, 2, (j0 + i) * 128:(j0 + i + 1) * 128],
                            qkT[:, h, qsl], True, True, mm_dtype)
                    if g == nq - 1:  # boundary quad: apply causal mask tiles
                        nc.vector.tensor_add(ps_quad[:], ps_quad[:], mask_sb[:])
                    nc.scalar.activation(pt[:, j0:j0 + 4, :], ps_quad[:],
                                         mybir.ActivationFunctionType.Exp, scale=SCALE)
                    for i in range(4):
                        _mm(nc, ps_o[:], v_sb[:, j0 + i, :], pt[:, j0 + i, :],
                            j0 + i == 0, j0 + i == 4 * nq - 1, mm_dtype)
                nc.vector.tensor_copy(attnT[:, qsl], ps_o[:])
                # softmax denominators: DVE-reduce probs over k-tiles, then a
                # ones-matmul partition reduction
                njt = 4 * nq
                red = sp2.tile([128, 512], F32, tag="red")
                nc.vector.tensor_reduce(
                    red[:], pt[:, 0:njt, :].rearrange("p a b -> p b a"),
                    axis=mybir.AxisListType.X, op=mybir.AluOpType.add)
                ps_cs = patt.tile([1, 512], F32, tag="ps_cs")
                _mm(nc, ps_cs[:], ones_sb[:], red[:], True, True, mm_dtype)
                nc.vector.reciprocal(recip_h[:, qsl], ps_cs[:])
            # normalize the whole head at once via a DRAM-bounce broadcast
            rb = dram.tile([1, 2048], F32, name=f"rb_{h}")
            nc.sync.dma_start(out=rb[:], in_=recip_h[:])
            bc = sp2.tile([128, 2048], F32, tag="bc")
            nc.sync.dma_start(out=bc[:], in_=rb.to_broadcast((128, 2048)))
            nc.vector.tensor_mul(attnT[:], attnT[:], bc[:])

        # ---------------- phase 3: output projection + ReduceScatter ----------------
        # 4 row-chunk collectives so the ReduceScatter overlaps the remaining
        # output-projection compute instead of being fully exposed at the end.
        for q4 in range(4):
            woacc = dram.tile([512, 2048], F32, name=f"woacc_{q4}")
            woacc_v = woacc.rearrange("(a p) n -> p a n", p=128)  # [128, 4, 2048]
            o_sb = opool.tile([128, 4, 2048], F32, tag="osb")
            for sti in range(4):
                st = 4 * q4 + sti
                ssl = slice(st * 128, (st + 1) * 128)
                ps_wo = patt.tile([128, 2048], F32, tag="pbig")
                for n in range(4):
                    nsl = slice(n * 512, (n + 1) * 512)
                    _mm(nc, ps_wo[:, nsl], attn0[:, ssl], wo_sb[:, 0, nsl],
                        True, False, mm_dtype)
                    _mm(nc, ps_wo[:, nsl], attn1[:, ssl], wo_sb[:, 1, nsl],
                        False, True, mm_dtype)
                nc.vector.tensor_copy(o_sb[:, sti, :], ps_wo[:])
            nc.sync.dma_start(out=woacc_v[:], in_=o_sb[:])

            rs_out = dram.tile([64, 2048], F32, name=f"rsout_{q4}")
            nc.gpsimd.collective_compute(
                "ReduceScatter",
                mybir.AluOpType.add,
                replica_groups=[list(range(NCORES))],
                ins=[woacc.opt()],
                outs=[rs_out.opt()],
            )
            nc.sync.dma_start(out=io["out"][q4 * 64:(q4 + 1) * 64, :], in_=rs_out[:])


def build(mm_dtype=None, repeat=1, num_devices=NCORES):
    mm_dtype = mm_dtype or MM_DTYPE
    key = (str(mm_dtype), repeat, num_devices)
    if key in _BUILD_CACHE:
        return _BUILD_CACHE[key]
    nc = bacc.Bacc("TRN2", target_bir_lowering=False, debug=False,
                   num_devices=num_devices)
    io = {
        "xT": nc.dram_tensor("xT", [D, S], F32, kind="ExternalInput").ap(),
        "wqkv": nc.dram_tensor("wqkv", [D, 512], F32, kind="ExternalInput").ap(),
        "wo": nc.dram_tensor("wo", [256, D], F32, kind="ExternalInput").ap(),
        "cosT": nc.dram_tensor("cosT", [64, S], F32, kind="ExternalInput").ap(),
        "sinT": nc.dram_tensor("sinT", [64, S], F32, kind="ExternalInput").ap(),
        "maskt": nc.dram_tensor("maskt", [128, 4, 512], F32, kind="ExternalInput").ap(),
        "ident": nc.dram_tensor("ident", [128, 128], F32, kind="ExternalInput").ap(),
        "out": nc.dram_tensor("out", [S // NCORES, D], F32, kind="ExternalOutput").ap(),
    }
    with tile.TileContext(nc) as tc:
        for _ in range(repeat):
            _emit_body(nc, tc, io, mm_dtype)
    nc.compile()
    _BUILD_CACHE[key] = nc
    return nc


def prepare_in_maps(x, wq, wk, wv, wo, freqs_cos, freqs_sin):
    x2d = np.asarray(x, dtype=np.float32).reshape(S, D)
    xT = np.ascontiguousarray(x2d.T)
    cosT = np.ascontiguousarray(np.asarray(freqs_cos, np.float32).T)
    sinT = np.ascontiguousarray(np.asarray(freqs_sin, np.float32).T)

    # even dims first, then odd dims (applied to q and k only)
    perm = np.concatenate([np.arange(0, HD, 2), np.arange(1, HD, 2)])

    # boundary mask tiles in scoresT layout: visible iff jl <= ql - 128*r
    jl = np.arange(128)[:, None]
    ql = np.arange(512)[None, :]
    maskt = np.empty((128, 4, 512), np.float32)
    for r in range(4):
        maskt[:, r, :] = np.where(jl <= ql - 128 * r, 0.0, NEG)

    ident = np.eye(128, dtype=np.float32)

    wq = np.asarray(wq, np.float32)
    wk = np.asarray(wk, np.float32)
    wv = np.asarray(wv, np.float32)
    wo = np.asarray(wo, np.float32)

    in_maps = []
    for c in range(NCORES):
        g = c // 2
        wq_c = wq[:, 2 * c * HD:(2 * c + 2) * HD].reshape(D, 2, HD)[:, :, perm].reshape(D, 2 * HD)
        wk_c = wk[:, g * HD:(g + 1) * HD][:, perm]
        wv_c = wv[:, g * HD:(g + 1) * HD]
        wqkv_c = np.ascontiguousarray(
            np.concatenate([wq_c, wk_c, wv_c], axis=1))  # [D, 512]
        wo_c = np.ascontiguousarray(wo[2 * c * HD:(2 * c + 2) * HD, :])  # [256, D]
        in_maps.append({
            "xT": xT,
            "wqkv": wqkv_c,
            "wo": wo_c,
            "cosT": cosT,
            "sinT": sinT,
            "maskt": maskt,
            "ident": ident,
        })
    return in_maps


def assemble_output(results):
    # RS chunk q4 covers global rows [512*q4, 512*q4+512); core c holds
    # rows [512*q4 + 64*c, 512*q4 + 64*c + 64) at out[64*q4 : 64*q4+64].
    shards = np.stack([results[c]["out"].reshape(4, 64, D) for c in range(NCORES)],
                      axis=1)  # [q4, core, 64, D]
    return np.ascontiguousarray(shards.reshape(S, D)).reshape(1, S, D)


def kernel(x, wq, wk, wv, wo, freqs_cos, freqs_sin, mask):
    nc = build()
    in_maps = prepare_in_maps(x, wq, wk, wv, wo, freqs_cos, freqs_sin)
    res = run_bass_kernel_spmd(nc, in_maps, core_ids=list(range(NCORES)))
    return assemble_output(res.results).astype(np.float32)
